# revision 1
# baseline (speedup 1.0000x reference)
"""Trainium2 Bass kernel for nn_LogisticRegression (embedding_lookup).

Reference computation (B=1024, S=200, V=50000, E=300):
    x1 = one-hot presence over vocab (duplicates set once)      [B, V]
    emb_mean = mean(emb_table[x], axis=1)                       [B, E]
    logits = concat([emb_mean, x1]) @ W.T + b                   [B, 1]
    out = sigmoid(logits)

Algebraic restructure (never materializes x1 / feats):
    t[v]     = emb_table[v] . W[0, :E] / S
    logit[i] = sum_j t[x[i,j]] + sum_j m[i,j] * W_voc[x[i,j]] + b
where m is the first-occurrence mask (dedup = the one-hot .set semantics).

Device plan (single NEFF, SPMD on 8 cores):
  phase 1 (vocab-sharded): core c computes t over its 6250-row table slice
           (reads 7.5MB of the 60MB table), pairs it with its W_voc slice.
  AllGather: 50KB/core (t, w) pair table -> full 401KB table on every core.
  phase 2 (batch-sharded): core c handles 128 batch rows. The (t, w) pair
           of token (p, j) is fetched with dma_gather at 256-byte block
           granularity (block = pair_idx // 32, fits int16; token slot
           s = j*128 + p lands on partition p = its batch row). Each
           gathered [128, 8, 64] slab is folded into per-row logits by a
           fused multiply-accumulate against a host-built one-hot weight
           slab (bf16, exact 0/1/m values):
              wv[p, j, 2*(pair%32)]   = 1      (selects t)
              wv[p, j, 2*(pair%32)+1] = m[p,j] (selects w, pre-masked)
  Finally sigmoid(logit + b) per row.

Empirical ground rules for this stack (established by direct HW tests):
  * indirect_dma_start runs ~10ns/descriptor serialized -> unusable here;
  * dma_gather works exactly, but <= 1024 indices per instruction;
  * tensor_tensor_reduce / tensor_scalar(accum_out) crash the compiled
    NEFF; scalar_tensor_tensor(accum_out) is exercised by USE_STT below.

Host side only shards tensors and precomputes integer index data (gather
block ids, one-hot selection weights, first-occurrence mask) from the int
token ids, then concatenates the per-core outputs.
"""

import sys

if "/opt/trn_rl_repo" not in sys.path:
    sys.path.insert(0, "/opt/trn_rl_repo")

# This image's antenv package lacks the optional axon_hooks module, but
# concourse.bass_utils imports it unconditionally on the BASS_TRACE path.
# Provide a compatible stub so tracing degrades gracefully instead of
# crashing; a harness may install a real hook via set_axon_ntff_profile_hook.
try:
    import antenv.axon_hooks  # noqa: F401
except ImportError:
    import types as _types

    import antenv as _antenv

    _hooks_mod = _types.ModuleType("antenv.axon_hooks")
    _hooks_mod._hook = None

    def _set_hook(h, _m=_hooks_mod):
        _m._hook = h

    def _get_hook(_m=_hooks_mod):
        return _m._hook

    _hooks_mod.set_axon_ntff_profile_hook = _set_hook
    _hooks_mod.get_axon_ntff_profile_hook = _get_hook
    sys.modules["antenv.axon_hooks"] = _hooks_mod
    _antenv.axon_hooks = _hooks_mod

import ml_dtypes
import numpy as np

from concourse import bacc, bass, mybir, tile
from concourse.bass_utils import run_bass_kernel_spmd

# Problem shapes (hardcoded per contract).
N_CORES = 8
B = 1024
S = 200
V = 50000
E = 300

RPC = B // N_CORES          # batch rows per core = 128
VPC = V // N_CORES          # vocab rows per core = 6250
KC = 49                     # free-dim columns of the per-core t layout
VPAD = KC * 128             # padded vocab rows per core = 6272
NPAIR = N_CORES * VPAD      # total (t, w) pairs after AllGather = 50176
TCHUNK = 7                  # table tiles per phase-1 DMA chunk
NCHUNK = KC // TCHUNK       # 7 chunks of 7 tiles

# phase-2 gather geometry
BPAIR = 32                  # pairs per 256B gather block
NBLK = NPAIR // BPAIR       # 1568 blocks
ESZ = 2 * BPAIR             # 64 f32 per block
GI = 1024                   # indices per dma_gather (HW limit)
NG = RPC * S // GI          # 25 gather instructions
JPG = GI // RPC             # 8 j-columns per gather

USE_STT = True              # fused (g*wv -> accum) on DVE

_BUILT = None
LAST_RUN = None  # BassKernelResults of the most recent launch (for harness)


def _build():
    f32 = mybir.dt.float32
    bf16 = mybir.dt.bfloat16
    i16 = mybir.dt.int16
    nc = bacc.Bacc("TRN2", target_bir_lowering=False, debug=False,
                   num_devices=N_CORES)

    tbl = nc.dram_tensor("tbl", [VPAD, E], f32, kind="ExternalInput")
    wemb = nc.dram_tensor("wemb", [1, E], f32, kind="ExternalInput")
    wvoc = nc.dram_tensor("wvoc", [128, KC], f32, kind="ExternalInput")
    gidx = nc.dram_tensor("gidx", [128, RPC * S // 16], i16, kind="ExternalInput")
    wv = nc.dram_tensor("wv", [RPC, S, ESZ], bf16, kind="ExternalInput")
    bias = nc.dram_tensor("bias", [1, 1], f32, kind="ExternalInput")
    outp = nc.dram_tensor("outp", [RPC, 1], f32, kind="ExternalOutput")

    with tile.TileContext(nc) as tc:
        with tc.tile_pool(name="dram", bufs=1, space="DRAM") as dram, \
             tc.tile_pool(name="sbuf", bufs=1) as sb1, \
             tc.tile_pool(name="ld", bufs=3) as ld, \
             tc.tile_pool(name="gbl", bufs=4) as gbl, \
             tc.tile_pool(name="scr", bufs=2) as scr:
            u_slice = dram.tile([VPAD, 2], f32)
            u_full = dram.tile([NBLK, ESZ], f32)

            # --- small input loads (overlap the table read) ---
            wemb_sb = sb1.tile([128, E], f32)
            nc.scalar.dma_start(wemb_sb[:], wemb.ap().partition_broadcast(128))
            # fold the 1/S of the sequence mean into the embedding weights
            nc.vector.tensor_scalar_mul(wemb_sb[:], wemb_sb[:], 1.0 / S)
            wvoc_sb = sb1.tile([128, KC], f32)
            nc.scalar.dma_start(wvoc_sb[:], wvoc.ap())
            gidx_sb = sb1.tile([128, RPC * S // 16], i16)
            nc.scalar.dma_start(gidx_sb[:], gidx.ap())
            wv_sb = sb1.tile([RPC, S, ESZ], bf16)
            nc.scalar.dma_start(wv_sb[:], wv.ap())
            b_sb = sb1.tile([128, 1], f32)
            nc.scalar.dma_start(b_sb[:], bias.ap().partition_broadcast(128))

            # u_sb[p, k, 0] = t[slice row 128k+p]/S ; u_sb[p, k, 1] = W_voc
            u_sb = sb1.tile([128, KC, 2], f32)

            # --- phase 1: t = tbl @ wemb / S, one mult + reduce per chunk ---
            wemb_bc = wemb_sb[:].unsqueeze(1).to_broadcast([128, TCHUNK, E])
            for ch in range(NCHUNK):
                rows = TCHUNK * 128
                chunk = ld.tile([128, TCHUNK, E], f32, tag="tblchunk")
                src = tbl.ap()[ch * rows:(ch + 1) * rows, :]
                nc.sync.dma_start(chunk[:], src.rearrange("(t p) e -> p t e", p=128))
                prod = scr.tile([128, TCHUNK, E], f32, tag="prod")
                nc.vector.tensor_tensor(
                    out=prod[:], in0=chunk[:], in1=wemb_bc,
                    op=mybir.AluOpType.mult)
                nc.vector.tensor_reduce(
                    out=u_sb[:, ch * TCHUNK:(ch + 1) * TCHUNK, 0],
                    in_=prod[:], axis=mybir.AxisListType.X,
                    op=mybir.AluOpType.add)
            nc.vector.tensor_copy(out=u_sb[:, :, 1], in_=wvoc_sb[:])
            nc.gpsimd.dma_start(u_slice[:], u_sb[:])

            # --- all-gather the (t, w) pair table ---
            nc.gpsimd.collective_compute(
                "AllGather",
                mybir.AluOpType.bypass,
                replica_groups=[list(range(N_CORES))],
                ins=[u_slice.opt()],
                outs=[u_full.opt()],
            )

            # --- phase 2: block-gather + fused extract/reduce per slab ---
            acc = sb1.tile([128, NG], f32)
            for k in range(NG):
                g = gbl.tile([128, JPG, ESZ], f32, tag="gblk")
                nc.gpsimd.dma_gather(
                    g[:], u_full[:],
                    gidx_sb[:, (GI // 16) * k:(GI // 16) * (k + 1)],
                    num_idxs=GI, num_idxs_reg=GI, elem_size=ESZ,
                )
                wv_k = wv_sb[:, JPG * k:JPG * (k + 1), :]
                if USE_STT:
                    po = scr.tile([128, JPG, ESZ], f32, tag="po")
                    nc.vector.scalar_tensor_tensor(
                        out=po[:], in0=g[:], scalar=1.0, in1=wv_k,
                        op0=mybir.AluOpType.mult, op1=mybir.AluOpType.mult,
                        accum_out=acc[:, k:k + 1])
                else:
                    po = scr.tile([128, JPG, ESZ], f32, tag="po")
                    nc.vector.tensor_tensor(
                        out=po[:], in0=g[:], in1=wv_k,
                        op=mybir.AluOpType.mult)
                    nc.vector.tensor_reduce(
                        out=acc[:, k:k + 1], in_=po[:],
                        axis=mybir.AxisListType.XY, op=mybir.AluOpType.add)

            logit = sb1.tile([128, 1], f32)
            nc.vector.tensor_reduce(
                out=logit[:], in_=acc[:], axis=mybir.AxisListType.X,
                op=mybir.AluOpType.add)
            res = sb1.tile([128, 1], f32)
            nc.scalar.activation(
                out=res[:], in_=logit[:],
                func=mybir.ActivationFunctionType.Sigmoid,
                bias=b_sb[:], scale=1.0)
            nc.scalar.dma_start(outp.ap(), res[:])

    nc.compile()
    return nc


def _first_occurrence_mask(xr: np.ndarray) -> np.ndarray:
    """m[i, j] = 1 iff x[i, j] does not appear at any k < j in row i."""
    eq = xr[:, :, None] == xr[:, None, :]          # [rows, S, S]
    dup = np.tril(eq, -1).any(axis=2)              # seen earlier in the row
    return ~dup


def kernel(x, emb_table, W, b):
    global _BUILT, LAST_RUN
    if _BUILT is None:
        _BUILT = _build()
    nc = _BUILT

    x = np.asarray(x)
    emb_table = np.ascontiguousarray(np.asarray(emb_table, dtype=np.float32))
    W = np.asarray(W, dtype=np.float32)
    b = np.asarray(b, dtype=np.float32)

    wemb = np.ascontiguousarray(W[:, :E])                  # [1, E]
    wv_full = W[0, E:]                                     # [V]
    bias_np = b.reshape(1, 1)

    in_maps = []
    for c in range(N_CORES):
        tbl = np.zeros((VPAD, E), dtype=np.float32)
        tbl[:VPC] = emb_table[c * VPC:(c + 1) * VPC]
        wvs = np.zeros(VPAD, dtype=np.float32)
        wvs[:VPC] = wv_full[c * VPC:(c + 1) * VPC]
        wvoc_sh = np.ascontiguousarray(wvs.reshape(KC, 128).T)  # [128, KC]

        xr = x[c * RPC:(c + 1) * RPC].astype(np.int64)          # [RPC, S]
        ct = xr // VPC
        r = xr - ct * VPC
        # global pair index (matches the phase-1 SBUF->DRAM flat layout)
        pidx = ct * VPAD + (r % 128) * KC + (r // 128)          # [RPC, S]
        m = _first_occurrence_mask(xr)                          # [RPC, S] bool

        # gather block ids, wrapped in 16 partitions, replicated x8
        blk = (pidx // BPAIR).astype(np.int16)                  # [RPC, S]
        s = np.arange(RPC * S)
        w16 = np.zeros((16, RPC * S // 16), dtype=np.int16)
        w16[s % 16, s // 16] = blk[s % RPC, s // RPC]           # slot s=(j*128+p)
        gidx_np = np.tile(w16, (8, 1))                          # [128, 1600]

        # one-hot extraction weights (exact in bf16)
        woff = (pidx % BPAIR) * 2                               # [RPC, S]
        wv_np = np.zeros((RPC, S, ESZ), dtype=ml_dtypes.bfloat16)
        rows = np.arange(RPC)[:, None]
        cols = np.arange(S)[None, :]
        wv_np[rows, cols, woff] = 1.0
        wv_np[rows, cols, woff + 1] = m.astype(ml_dtypes.bfloat16)

        in_maps.append({
            "tbl": tbl,
            "wemb": wemb,
            "wvoc": wvoc_sh,
            "gidx": gidx_np,
            "wv": wv_np,
            "bias": bias_np,
        })

    LAST_RUN = run_bass_kernel_spmd(nc, in_maps, core_ids=list(range(N_CORES)))
    out = np.concatenate(
        [LAST_RUN.results[c]["outp"].reshape(RPC) for c in range(N_CORES)]
    )
    return out.reshape(B, 1)



# revision 2
# speedup vs baseline: 1.1222x; 1.1222x over previous
"""Trainium2 Bass kernel v2.2 for nn_LogisticRegression (embedding_lookup).

Algebra: logit[i] = sum_j t[x[i,j]] + sum_j m[i,j]*w[x[i,j]] + b
with t[v] = emb_table[v] . W[0,:E] / S, w = W[0, E:], m = first-occurrence.

Device plan (single NEFF, SPMD on 8 cores):
  phase 1 (vocab-sharded): core c computes t over its 6250-row slice
      (gpsimd mult + DVE free-reduce, hidden under the launch barrier) and
      the interleaved pair columns u2 = (t, t+w) bf16, split in two halves.
  Two AllGathers (early-triggered so the CC handshake hides under the
      launch barrier): first cols [0,25) = phi' [0,200), then cols [25,49)
      = phi' [200,392), with phi'(v) = col*8 + core (col-major interleave),
      so each gather delivers a contiguous phi' range every core can start
      consuming immediately.
  phase 2 (batch-sharded, per-row phi'-sorted tokens): tile k holds every
      row's k-th smallest-phi' token. One PE matmul per tile with a
      host-built fp8 one-hot stationary picks each token's pi row from the
      U window [w0_k, w0_k+wid_k) (variable width); tiles whose window
      straddles phi'=200 issue two matmuls (one per U half) into disjoint
      column ranges of the same PSUM tile. Tiles are packed two per PSUM
      bank; one fused STT against a host fp8 one-hot wv extracts
      psum[slot, fstar] (fstar = 2*(phi'-w0) + m; col 2phi holds t,
      2phi+1 holds t+w) and accumulates per row.
  Final: row-reduce acc, sigmoid(logit + b), write [128, 1].

The window schedule is computed from x at first kernel() call and baked
into the NEFF; the host asserts every token falls inside its window.
"""

import sys

if "/opt/trn_rl_repo" not in sys.path:
    sys.path.insert(0, "/opt/trn_rl_repo")

try:
    import antenv.axon_hooks  # noqa: F401
except ImportError:
    import types as _types

    import antenv as _antenv

    _hooks_mod = _types.ModuleType("antenv.axon_hooks")
    _hooks_mod._hook = None

    def _set_hook(h, _m=_hooks_mod):
        _m._hook = h

    def _get_hook(_m=_hooks_mod):
        return _m._hook

    _hooks_mod.set_axon_ntff_profile_hook = _set_hook
    _hooks_mod.get_axon_ntff_profile_hook = _get_hook
    sys.modules["antenv.axon_hooks"] = _hooks_mod
    _antenv.axon_hooks = _hooks_mod

import ml_dtypes
import numpy as np

from concourse import bacc, bass, mybir, tile
from concourse.bass_utils import run_bass_kernel_spmd

N_CORES = 8
B = 1024
S = 200
V = 50000
E = 300
RPC = B // N_CORES          # 128 batch rows per core
VPC = V // N_CORES          # 6250 vocab rows per core
KC = 49                     # pair columns per core
VPAD = KC * 128
NPHI = N_CORES * KC         # 392 phi' values
T = S                       # tiles = tokens per row
CA = 25                     # cols in AllGather A (phi' < 200)
CB = KC - CA                # cols in AllGather B
PHIA = CA * N_CORES         # 200
TCH = 7

_BUILT = None
LAST_RUN = None


def _build(w0s, wids):
    f32 = mybir.dt.float32
    bf16 = mybir.dt.bfloat16
    fp8 = mybir.dt.float8e4
    nc = bacc.Bacc("TRN2", target_bir_lowering=False, debug=False,
                   num_devices=N_CORES)

    Ws = [2 * w for w in wids]
    offs = np.concatenate([[0], np.cumsum(Ws)])
    WSUM = int(offs[-1])

    tbl = nc.dram_tensor("tbl", [VPAD, E], f32, kind="ExternalInput")
    wemb = nc.dram_tensor("wemb", [1, E], f32, kind="ExternalInput")
    wvoc = nc.dram_tensor("wvoc", [128, KC], f32, kind="ExternalInput")
    oh = nc.dram_tensor("oh", [128, T * 128], fp8, kind="ExternalInput")
    wv = nc.dram_tensor("wv", [128, WSUM], fp8, kind="ExternalInput")
    bias = nc.dram_tensor("bias", [1, 1], f32, kind="ExternalInput")
    outp = nc.dram_tensor("outp", [RPC, 1], f32, kind="ExternalOutput")

    with tile.TileContext(nc) as tc:
        with tc.tile_pool(name="dram", bufs=1, space="DRAM") as dram, \
             tc.tile_pool(name="sbuf", bufs=1) as sb1, \
             tc.tile_pool(name="ld", bufs=3) as ld, \
             tc.tile_pool(name="psum", bufs=8, space="PSUM") as pp, \
             tc.tile_pool(name="scr", bufs=4) as scr:
            u_sl_a = dram.tile([128, 2 * CA], bf16)
            u_sl_b = dram.tile([128, 2 * CB], bf16)
            u_fl_a = dram.tile([N_CORES * 128, 2 * CA], bf16)
            u_fl_b = dram.tile([N_CORES * 128, 2 * CB], bf16)

            # --- small loads (scalar queue; table chunks go on sync) ---
            wemb_sb = sb1.tile([128, E], f32)
            nc.scalar.dma_start(wemb_sb[:], wemb.ap().partition_broadcast(128))
            nc.vector.tensor_scalar_mul(wemb_sb[:], wemb_sb[:], 1.0 / S)
            wvoc_sb = sb1.tile([128, KC], f32)
            nc.scalar.dma_start(wvoc_sb[:], wvoc.ap())
            oh_sb = sb1.tile([128, T, 128], fp8)
            nc.scalar.dma_start(oh_sb[:], oh.ap())
            wv_sb = sb1.tile([128, WSUM], fp8)
            nc.scalar.dma_start(wv_sb[:], wv.ap())
            b_sb = sb1.tile([128, 1], f32)
            nc.scalar.dma_start(b_sb[:], bias.ap().partition_broadcast(128))

            # --- phase 1: t = tbl @ wemb / S, contiguous p-major chunks.
            # Emitted in two halves so the first AllGather trigger issues on
            # the in-order gpsimd stream right after chunk 3.
            wemb_bc = wemb_sb[:].unsqueeze(1).to_broadcast([128, TCH, E])
            u_tA = sb1.tile([128, 4 * TCH], f32)
            u_tB = sb1.tile([128, 3 * TCH], f32)

            def p1_chunk(ch):
                rows = TCH * 128
                chunk = ld.tile([128, TCH, E], f32, tag="tblchunk")
                src = tbl.ap()[ch * rows:(ch + 1) * rows, :]
                nc.sync.dma_start(
                    chunk[:], src.rearrange("(p t) e -> p t e", t=TCH))
                prod = scr.tile([128, TCH, E], f32, tag="prod")
                nc.gpsimd.tensor_tensor(
                    out=prod[:], in0=chunk[:], in1=wemb_bc,
                    op=mybir.AluOpType.mult)
                dst = (u_tA[:, ch * TCH:(ch + 1) * TCH] if ch < 4
                       else u_tB[:, (ch - 4) * TCH:(ch - 3) * TCH])
                nc.vector.tensor_reduce(
                    out=dst, in_=prod[:], axis=mybir.AxisListType.X,
                    op=mybir.AluOpType.add)

            for ch in range(4):
                p1_chunk(ch)

            # interleaved (t, t+w) bf16 pair columns for phi' < 200
            # (built on gpsimd so the gather chain stays on one in-order queue)
            u2a = sb1.tile([128, CA, 2], bf16)
            nc.gpsimd.tensor_copy(out=u2a[:, :, 0], in_=u_tA[:, :CA])
            nc.gpsimd.tensor_tensor(
                out=u2a[:, :, 1], in0=u_tA[:, :CA], in1=wvoc_sb[:, :CA],
                op=mybir.AluOpType.add)
            nc.gpsimd.dma_start(u_sl_a[:], u2a[:])
            nc.gpsimd.collective_compute(
                "AllGather", mybir.AluOpType.bypass,
                replica_groups=[list(range(N_CORES))],
                ins=[u_sl_a.opt()], outs=[u_fl_a.opt()])

            for ch in range(4, KC // TCH):
                p1_chunk(ch)

            u2b = sb1.tile([128, CB, 2], bf16)
            nc.gpsimd.tensor_copy(out=u2b[:, :3, 0], in_=u_tA[:, CA:])
            nc.gpsimd.tensor_tensor(
                out=u2b[:, :3, 1], in0=u_tA[:, CA:], in1=wvoc_sb[:, CA:28],
                op=mybir.AluOpType.add)
            nc.gpsimd.tensor_copy(out=u2b[:, 3:, 0], in_=u_tB[:])
            nc.gpsimd.tensor_tensor(
                out=u2b[:, 3:, 1], in0=u_tB[:], in1=wvoc_sb[:, 28:],
                op=mybir.AluOpType.add)
            nc.gpsimd.dma_start(u_sl_b[:], u2b[:])
            nc.gpsimd.collective_compute(
                "AllGather", mybir.AluOpType.bypass,
                replica_groups=[list(range(N_CORES))],
                ins=[u_sl_b.opt()], outs=[u_fl_b.opt()])

            # U halves in per-half c-major layout [p, c, col, 2] so the
            # re-read is 100B-contiguous per (p, c); phi'' = c*CA + col (A),
            # 200 + c*CB + (col-CA) (B)
            U_a = sb1.tile([128, 2 * CA * N_CORES], bf16)
            nc.sync.dma_start(
                U_a[:].rearrange("p (c f) -> p c f", c=N_CORES),
                u_fl_a[:].rearrange("(c p) f -> p c f", p=128))
            U_b = sb1.tile([128, 2 * CB * N_CORES], bf16)
            nc.sync.dma_start(
                U_b[:].rearrange("p (c f) -> p c f", c=N_CORES),
                u_fl_b[:].rearrange("(c p) f -> p c f", p=128))

            def rhs_parts(k):
                w0, wd = w0s[k], wids[k]
                if w0 + wd <= PHIA:
                    return [(U_a, 2 * w0, 2 * wd)]
                if w0 >= PHIA:
                    return [(U_b, 2 * (w0 - PHIA), 2 * wd)]
                wa = PHIA - w0
                return [(U_a, 2 * w0, 2 * wa), (U_b, 0, 2 * (wd - wa))]

            # --- phase 2: paired tiles, one-hot matmul + fused extraction ---
            acc = sb1.tile([128, T // 2], f32)
            for g in range(0, T, 2):
                k0, k1 = g, g + 1
                wA, wB = Ws[k0], Ws[k1]
                ps = pp.tile([128, wA + wB], f32, tag="ps")
                base = 0
                for k, wk in ((k0, wA), (k1, wB)):
                    off = 0
                    for (Uh, src0, cols) in rhs_parts(k):
                        nc.tensor.matmul(
                            ps[:, base + off:base + off + cols],
                            oh_sb[:, k, :], Uh[:, src0:src0 + cols])
                        off += cols
                    base += wk
                po = scr.tile([128, wA + wB], f32, tag="po")
                nc.vector.scalar_tensor_tensor(
                    out=po[:], in0=ps[:], scalar=1.0,
                    in1=wv_sb[:, int(offs[k0]):int(offs[k0]) + wA + wB],
                    op0=mybir.AluOpType.mult, op1=mybir.AluOpType.mult,
                    accum_out=acc[:, g // 2:g // 2 + 1])

            logit = sb1.tile([128, 1], f32)
            nc.vector.tensor_reduce(
                out=logit[:], in_=acc[:], axis=mybir.AxisListType.X,
                op=mybir.AluOpType.add)
            res = sb1.tile([128, 1], f32)
            nc.scalar.activation(
                out=res[:], in_=logit[:],
                func=mybir.ActivationFunctionType.Sigmoid,
                bias=b_sb[:], scale=1.0)
            nc.scalar.dma_start(outp.ap(), res[:])

    nc.compile()
    return nc


def _first_occurrence_mask(xr: np.ndarray) -> np.ndarray:
    eq = xr[:, :, None] == xr[:, None, :]
    dup = np.tril(eq, -1).any(axis=2)
    return ~dup


def _coords(xall):
    """phi'' (per-half c-major) and pi for the p-major chunk layout."""
    c = xall // VPC
    r = xall % VPC
    col = 7 * (r // (TCH * 128)) + (r % TCH)
    pi = (r % (TCH * 128)) // TCH
    phi = np.where(col < CA, c * CA + col, PHIA + c * CB + (col - CA))
    return phi, pi


def kernel(x, emb_table, W=None, b=None, **kw):
    global _BUILT, LAST_RUN
    if W is None:
        W = kw.pop("W")
    if b is None:
        b = kw.pop("b")

    x = np.asarray(x)
    emb_table = np.ascontiguousarray(np.asarray(emb_table, dtype=np.float32))
    Wf = np.asarray(W, dtype=np.float32)
    b = np.asarray(b, dtype=np.float32)

    wemb = np.ascontiguousarray(Wf[:, :E])
    wv_full = Wf[0, E:]
    bias_np = b.reshape(1, 1)

    xall = x.astype(np.int64)
    phi_all, _ = _coords(xall)
    phis_all = np.sort(phi_all, axis=1)
    lo = phis_all.min(axis=0)
    hi = phis_all.max(axis=0)
    w0s = [int(v) for v in lo]
    wids = [int(h - l + 1) for l, h in zip(lo, hi)]

    key = (tuple(w0s), tuple(wids))
    if _BUILT is None or _BUILT[1] != key:
        _BUILT = (_build(w0s, wids), key)
    nc = _BUILT[0]

    Ws = [2 * w for w in wids]
    offs = np.concatenate([[0], np.cumsum(Ws)]).astype(np.int64)
    WSUM = int(offs[-1])

    rows_i = np.arange(RPC)[:, None]
    cols_k = np.arange(T)[None, :]
    in_maps = []
    for c in range(N_CORES):
        tblc = np.zeros((VPAD, E), dtype=np.float32)
        tblc[:VPC] = emb_table[c * VPC:(c + 1) * VPC]
        # wvoc[pi, col] = w of local row r with that (pi, col) in p-major map
        wvs = np.zeros(VPAD, dtype=np.float32)
        wvs[:VPC] = wv_full[c * VPC:(c + 1) * VPC]
        # local r = 896*(col//7) + 7*pi + (col%7)
        pi_g, col_g = np.meshgrid(np.arange(128), np.arange(KC), indexing="ij")
        rloc = 896 * (col_g // 7) + 7 * pi_g + (col_g % 7)
        wvoc_sh = np.ascontiguousarray(wvs[rloc]).astype(np.float32)

        xr = xall[c * RPC:(c + 1) * RPC]
        phi, pi = _coords(xr)
        m = _first_occurrence_mask(xr)

        order = np.argsort(phi, axis=1, kind="stable")
        phi_s = np.take_along_axis(phi, order, axis=1)
        pi_s = np.take_along_axis(pi, order, axis=1)
        m_s = np.take_along_axis(m, order, axis=1)

        fstar = 2 * (phi_s - np.asarray(w0s)[None, :]) + m_s
        assert (fstar >= 0).all() and (fstar < np.asarray(Ws)[None, :]).all()

        oh_np = np.zeros((128, T, 128), dtype=ml_dtypes.float8_e4m3fn)
        oh_np[pi_s[rows_i, cols_k], cols_k, rows_i] = 1.0

        wv_np = np.zeros((128, WSUM), dtype=ml_dtypes.float8_e4m3fn)
        wv_np[rows_i, offs[None, :T] + fstar] = 1.0

        in_maps.append({
            "tbl": tblc,
            "wemb": wemb,
            "wvoc": wvoc_sh,
            "oh": oh_np.reshape(128, T * 128),
            "wv": wv_np,
            "bias": bias_np,
        })

    LAST_RUN = run_bass_kernel_spmd(nc, in_maps, core_ids=list(range(N_CORES)))
    out = np.concatenate(
        [LAST_RUN.results[c]["outp"].reshape(RPC) for c in range(N_CORES)]
    )
    return out.reshape(B, 1)
